# revision 21
# baseline (speedup 1.0000x reference)
"""Trainium2 Bass kernel for causal self-attention (muP scaling).

Full-input contract: kernel(**inputs) takes the complete tensors and returns
the complete [B, T, C] output. Work splits over 8 NeuronCores as
(batch b = core//2) x (head-group g = core%2, 8 heads each); each core
produces a partial [T, C] output (its 8 heads through the matching 512-row
slice of w_proj) and the host sums the two partials per batch plus b_proj.

Key speed trick vs the bf16 baseline: the q/k path runs in fp8-e4m3 with
DoubleRow matmuls (2 fp8 MACs per PE cell per cycle):
  - phase 1 q,k projection: x and w_qk are DR-packed on the host as
    [K/2, 2, N] plane pairs; 4 DR matmuls replace 8 bf16 matmuls.
    w_qk is prescaled x16 so its sigma=1/32 values clear the e4m3
    denormal floor; q,k come out x16 which the exp scale folds away.
  - score matmuls: q,k are re-quantized to e4m3 (after an f32 drain) and
    the 64-dim head contraction is split as 2 planes of 32 partitions,
    so each score matmul is DR too. Four heads share one [128,...] tile
    at partition offsets 0/32/64/96 (PE tile_position handles the rest).
    A [32, T] plane-shuffle DMA per quadrant builds this layout off the
    critical path.
The value path (v projection, attention-weighted sum, output projection)
stays bf16: e4m3 there costs 3-6e-2 relative error (gate is 2e-2), while
fp8 on q/k only costs ~7e-3 because muP logits are tiny (sigma 0.125) and
softmax squashes operand noise.

Attention core (unchanged from the bf16 baseline): head PAIRS share one
[128, 2*512] PSUM tile (even head segment 0, odd segment 1) drained by a
single ScalarE exp (2-segment strided AP, scale 1/(64*256), no
max-subtraction -- logits cannot overflow). Causal masking multiplies a 0/1
triangle into diagonal-crossing tiles only; fully-invalid tiles are never
computed. attT-out[d, tq] accumulates v_aug.T @ expT where v_aug carries an
appended ones column, so row 64 of the accumulator is the softmax
denominator for free; reciprocal + gpsimd partition-broadcast + one fused
multiply normalize while copying PSUM->SBUF into the [c, t] layout the
final projection wants as its stationary operand. Attention blocks iterate
tq-block-outer so each finished tq column group's output projection
interleaves with the next block's attention.
"""

import sys

if "/opt/trn_rl_repo" not in sys.path:
    sys.path.insert(0, "/opt/trn_rl_repo")

import numpy as np
import ml_dtypes

import concourse.bass as bass
import concourse.mybir as mybir
import concourse.tile as tile
from concourse import bacc
from concourse.masks import make_upper_triangular

# Problem shape (hardcoded per contract).
B, T, C, H = 4, 2048, 1024, 16
HD = C // H            # 64
N_CORES = 8
HG = H // 2            # 8 heads per core
GC = HG * HD           # 512 columns of q/k/v per core
P = 128                # SBUF partitions
CT = C // P            # 8 contraction tiles over C
TP = CT // 2           # 4 DoubleRow contraction pairs
TT = T // P            # 16 time tiles of 128
QB = 4                 # tq blocks
QW = T // QB           # 512 wide
KT = T // P            # 16 tk tiles
WSCALE = 16.0          # host prescale on w_qk (and b_qk) before e4m3

_bf16np = ml_dtypes.bfloat16
F32 = mybir.dt.float32
BF16 = mybir.dt.bfloat16
FP8 = mybir.dt.float8e4
_fp8np = mybir.dt.np(FP8)
DR = mybir.MatmulPerfMode.DoubleRow

_COMPILED = None


def _build_nc(reps=1, phases=(1, 2, 3), pipeline=False):
    nc = bacc.Bacc("TRN2", target_bir_lowering=False, debug=False,
                   num_devices=N_CORES)

    xT = nc.dram_tensor("xT", [C, T], BF16, kind="ExternalInput").ap()
    x_dr = nc.dram_tensor("x_dr", [C // 2, 2 * T], FP8, kind="ExternalInput").ap()
    w_dr = nc.dram_tensor("w_dr", [C // 2, 2 * 2 * GC], FP8, kind="ExternalInput").ap()
    w_v = nc.dram_tensor("w_v", [C, GC], BF16, kind="ExternalInput").ap()
    b_qk = nc.dram_tensor("b_qk", [2 * GC], F32, kind="ExternalInput").ap()
    b_v = nc.dram_tensor("b_v", [GC], F32, kind="ExternalInput").ap()
    w_pr = nc.dram_tensor("w_pr", [GC, C], BF16, kind="ExternalInput").ap()
    y = nc.dram_tensor("y", [T, C], F32, kind="ExternalOutput").ap()

    with tile.TileContext(nc) as tc:
        for _ in range(reps):
            _emit(nc, tc, xT, x_dr, w_dr, w_v, b_qk, b_v, w_pr, y,
                  phases=phases, pipeline=pipeline)
    nc.finalize()
    return nc


def _emit(nc, tc, xT, x_dr, w_dr, w_v, b_qk, b_v, w_pr, y,
          phases=(1, 2, 3), pipeline=False):
    from contextlib import ExitStack

    ctx = ExitStack()
    with ctx:
        persist = ctx.enter_context(tc.tile_pool(name="persist", bufs=1))

        # ---- constants -------------------------------------------------
        tri = persist.tile([P, P], BF16, tag="tri")     # 0/1, 1 iff j >= i
        make_upper_triangular(nc, tri[:, :], val=1.0, diag=True)

        bqk_sb = persist.tile([P, CT], F32, tag="bqk")  # [128, 8] col jt
        nc.sync.dma_start(
            out=bqk_sb[:, :],
            in_=bass.AP(tensor=b_qk.tensor, offset=0, ap=[[1, P], [P, CT]]),
        )
        bv_sb = persist.tile([P, GC], F32, tag="bv")
        nc.gpsimd.dma_start(
            out=bv_sb[:, :],
            in_=bass.AP(tensor=b_v.tensor, offset=0, ap=[[0, P], [1, GC]]),
        )

        # ---- persistent activation buffers ----------------------------
        # q8p/k8p: 2 heads per tile -- head h at partitions 64*(h%2)+0:32
        # (AP base partitions only encode 0/32/64); free layout [plane, t]
        # (plane = head-dim 32-half, the DoubleRow contraction pair)
        q8p = [persist.tile([P, 2, T], FP8, name=f"q8p{i}", tag=f"q8p{i}")
               for i in range(4)]
        k8p = [persist.tile([P, 2, T], FP8, name=f"k8p{i}", tag=f"k8p{i}")
               for i in range(4)]
        v_sb = [persist.tile([P, HG, HD + 1], BF16, name=f"v{t}", tag=f"v{t}")
                for t in range(TT)]
        att = [persist.tile([P, T], BF16, name=f"att{j}", tag=f"att{j}")
               for j in range(CT // 2)]
        wpr = [persist.tile([P, C], BF16, name=f"wpr{j}", tag=f"wpr{j}")
               for j in range(CT // 2)]
        for ct in range(CT // 2):
            nc.sync.dma_start(out=wpr[ct][:, :], in_=w_pr[ct * P:(ct + 1) * P, :])

        # ---- phase-1 inputs (pools stay open: phase 1 is interleaved
        # into phase 2 so the PE has projection work during exp lag) -----
        xdrp = ctx.enter_context(tc.tile_pool(name="xdr", bufs=1))
        xp = ctx.enter_context(tc.tile_pool(name="xT", bufs=1))
        wdrp = ctx.enter_context(tc.tile_pool(name="wdr", bufs=1))
        qk8p = ctx.enter_context(tc.tile_pool(name="qk8t", bufs=1))
        wvp = ctx.enter_context(tc.tile_pool(name="wv", bufs=1))
        expp = ctx.enter_context(tc.tile_pool(name="expp", bufs=16))
        nrm = ctx.enter_context(tc.tile_pool(name="nrm", bufs=2))
        yp = ctx.enter_context(tc.tile_pool(name="ysb", bufs=2))
        # PSUM budget (8 banks): misc 2x1 + scores 2x2 + acc 2x1 = 8
        misc = ctx.enter_context(tc.tile_pool(name="ps_m", bufs=2, space="PSUM"))
        pss = ctx.enter_context(tc.tile_pool(name="ps_s", bufs=2, space="PSUM"))
        pso = ctx.enter_context(tc.tile_pool(name="ps_o", bufs=2, space="PSUM"))

        xdr = [xdrp.tile([P, 2, T], FP8, name=f"xdr{tp}", tag=f"xdr{tp}")
               for tp in range(TP)]
        xts = [xp.tile([P, T], BF16, name=f"xT{ct}", tag=f"xT{ct}")
               for ct in range(CT)]
        wdr = [wdrp.tile([P, 2, 2 * GC], FP8, name=f"wdr{tp}",
                         tag=f"wdr{tp}") for tp in range(TP)]
        qk8 = [qk8p.tile([P, T], FP8, name=f"qk8_{jt}", tag=f"qk8_{jt}")
               for jt in range(CT)]
        wvts = [wvp.tile([P, GC], BF16, name=f"wv{ct}", tag=f"wv{ct}")
                for ct in range(CT)]
        x_dr3 = x_dr.rearrange("k (s t) -> k s t", s=2)
        w_dr3 = w_dr.rearrange("k (s n) -> k s n", s=2)
        # spread input loads over four DGE queues (a single queue runs its
        # DMAs back-to-back: ~9MB of inputs would serialize for ~28us);
        # the qk operands (wdr+xdr, needed by the first matmul) go first
        queues = [nc.sync, nc.scalar, nc.gpsimd]
        qi = 0

        def load(out, in_):
            nonlocal qi
            queues[qi % 3].dma_start(out=out, in_=in_)
            qi += 1

        for tp in range(TP):
            load(wdr[tp][:, :, :], w_dr3[tp * P:(tp + 1) * P, :, :])
            load(xdr[tp][:, :, :], x_dr3[tp * P:(tp + 1) * P, :, :])
        for ct in range(CT):
            load(xts[ct][:, :], xT[ct * P:(ct + 1) * P, :])
            load(wvts[ct][:, :], w_v[ct * P:(ct + 1) * P, :])

        def emit_qk(jt):
            """q/k projection for one 128-dim output tile, fp8 DoubleRow."""
            for tb in range(QB):
                ps = misc.tile([P, QW], F32, tag="ps1")
                for tp in range(TP):
                    nc.tensor.matmul(
                        ps[:, :],
                        wdr[tp][:, :, jt * P:(jt + 1) * P],
                        xdr[tp][:, :, tb * QW:(tb + 1) * QW],
                        start=(tp == 0), stop=(tp == TP - 1),
                        perf_mode=DR,
                    )
                # drain on ScalarE: it idles during phase 1 (the short qb0
                # exps), while DVE is saturated by v drains there
                nc.scalar.activation(
                    out=qk8[jt][:, tb * QW:(tb + 1) * QW],
                    in_=ps[:, :],
                    func=mybir.ActivationFunctionType.Identity,
                    bias=bqk_sb[:, jt:jt + 1],
                )
            # plane-shuffle into the 2-heads-per-tile score layout
            dstset = q8p if jt < 4 else k8p
            jj = jt if jt < 4 else jt - 4
            for a in range(4):
                p0 = 64 * (a // 2)
                eng = nc.gpsimd if a % 2 else nc.sync
                eng.dma_start(
                    out=dstset[jj][p0:p0 + 32, a % 2, :],
                    in_=qk8[jt][32 * a:32 * a + 32, :],
                )

        def emit_v(tg):
            """v projection for time tiles 4*tg .. 4*tg+3 (bf16)."""
            for i in range(4):
                tt = tg * 4 + i
                ps = misc.tile([P, GC], F32, tag="ps1")
                for ct in range(CT):
                    nc.tensor.matmul(
                        ps[:, :],
                        xts[ct][:, tt * P:(tt + 1) * P],
                        wvts[ct][:, :],
                        start=(ct == 0), stop=(ct == CT - 1),
                    )
                nc.vector.tensor_add(
                    out=v_sb[tt][:, :, 0:HD],
                    in0=ps[:, :].rearrange("p (h e) -> p h e", e=HD),
                    in1=bv_sb[:, :].rearrange("p (h e) -> p h e", e=HD),
                )
                nc.vector.memset(v_sb[tt][:, :, HD:HD + 1], 1.0)

        def head_ap(tset, h, c0, c1):
            # [32, 2, c1-c0] view of head h's fp8 planes
            p0 = 64 * (h % 2)
            return tset[h // 2][p0:p0 + 32, :, c0:c1]

        def emit_scores(hp, q0, kt, off, crossing):
            n = QW - off
            ex = expp.tile([P, 2 * QW], BF16, tag="exp")
            ps = pss.tile([P, 2 * QW], F32, tag="scores")
            for half in range(2):
                h = 2 * hp + half
                nc.tensor.matmul(
                    ps[:, half * QW:half * QW + n],
                    head_ap(k8p, h, kt * P, (kt + 1) * P),
                    head_ap(q8p, h, q0 + off, q0 + QW),
                    start=True, stop=True,
                    perf_mode=DR,
                )
            # one exp over both heads: 2-segment strided view
            ps2 = ps[:, :].rearrange("p (s q) -> p s q", s=2)
            ex2 = ex[:, :].rearrange("p (s q) -> p s q", s=2)
            nc.scalar.activation(
                out=ex2[:, :, 0:n], in_=ps2[:, :, 0:n],
                func=mybir.ActivationFunctionType.Exp,
                scale=1.0 / (HD * WSCALE * WSCALE),
            )
            if crossing:
                # causal triangle on the diagonal 128 columns, split
                # across DVE and gpsimd so neither becomes the chain link
                nc.vector.tensor_mul(
                    out=ex[:, 0:P], in0=ex[:, 0:P], in1=tri[:, :])
                nc.gpsimd.tensor_mul(
                    out=ex[:, QW:QW + P], in0=ex[:, QW:QW + P],
                    in1=tri[:, :])
            return ex

        def emit_av(st, i):
            (hp, q0, accs, exps) = st
            kt, off, n, ex = exps[i]
            last = i == len(exps) - 1
            nc.tensor.matmul(
                accs[0][0:HD + 1, off:QW],
                v_sb[kt][:, 2 * hp, :],
                ex[:, 0:n],
                start=(i == 0), stop=last,
                skip_group_check=True,
            )
            nc.tensor.matmul(
                accs[1][0:HD + 1, off:QW],
                v_sb[kt][:, 2 * hp + 1, :],
                ex[:, QW:QW + n],
                start=(i == 0), stop=last,
                skip_group_check=True,
            )

        def emit_norm(st):
            (hp, q0, accs, exps) = st
            for half, acc in ((0, accs[0]), (1, accs[1])):
                r0 = half * HD
                rec = nrm.tile([P, QW], F32, tag="rec")
                nc.vector.reciprocal(out=rec[0:1, :],
                                     in_=acc[HD:HD + 1, :])
                bc = nrm.tile([P, QW], F32, tag="bc")
                nc.gpsimd.partition_broadcast(
                    bc[0:HD, :], rec[0:1, :], channels=HD)
                nc.vector.tensor_mul(
                    out=att[hp][r0:r0 + HD, q0:q0 + QW],
                    in0=acc[0:HD, :],
                    in1=bc[0:HD, :],
                )

        pend = [None]  # previous block whose AVs are deferred

        def emit_block(hp, qb):
            """Scores+exp for this block; the PREVIOUS block's AV matmuls
            interleave tile-by-tile so the PE never waits on ScalarE."""
            q0 = qb * QW
            tiles = [(kt, 0, False) for kt in range(4 * qb)]
            tiles += [(4 * qb + a, P * a, True) for a in range(4)]
            acc_e = pso.tile([P, QW], F32, name=f"acc_e{hp}_{qb}", tag="acc")
            acc_o = pso.tile([P, QW], F32, name=f"acc_o{hp}_{qb}", tag="acc")
            exps = []
            np_prev = len(pend[0][3]) if pend[0] is not None else 0
            for i, (kt, off, crossing) in enumerate(tiles):
                ex = emit_scores(hp, q0, kt, off, crossing)
                exps.append((kt, off, QW - off, ex))
                if pend[0] is not None:
                    lo = i * np_prev // len(tiles)
                    hi = (i + 1) * np_prev // len(tiles)
                    for j in range(lo, hi):
                        emit_av(pend[0], j)
            if pend[0] is not None:
                emit_norm(pend[0])
            pend[0] = (hp, q0, (acc_e, acc_o), exps)

        def flush_block():
            if pend[0] is not None:
                for j in range(len(pend[0][3])):
                    emit_av(pend[0], j)
                emit_norm(pend[0])
                pend[0] = None

        def emit_proj_group(tts):
            if 3 not in phases:
                return
            for tt in tts:
                ysb = yp.tile([P, C], F32, tag="y")
                for nb in range(2):
                    ps = misc.tile([P, QW], F32, tag="ps1")
                    for ct in range(CT // 2):
                        nc.tensor.matmul(
                            ps[:, :],
                            att[ct][:, tt * P:(tt + 1) * P],
                            wpr[ct][:, nb * QW:(nb + 1) * QW],
                            start=(ct == 0), stop=(ct == CT // 2 - 1),
                        )
                    nc.vector.tensor_copy(
                        out=ysb[:, nb * QW:(nb + 1) * QW], in_=ps[:, :])
                nc.sync.dma_start(out=y[tt * P:(tt + 1) * P, :], in_=ysb[:, :])

        # ---- interleaved schedule -------------------------------------
        # qk tiles feed head-pair hp from jt=hp (q) and jt=4+hp (k);
        # qb-k attention needs v time-tiles <= 4k+3 (group tg<=k). Emit
        # phase-1 chunks between attention blocks so the PE always has
        # dense projection work while ScalarE chews through the exps.
        if 1 in phases and 2 in phases:
            emit_qk(0); emit_qk(4)
            emit_v(0)
            emit_block(0, 0)
            emit_qk(1); emit_qk(5)
            emit_block(1, 0)
            emit_qk(2); emit_qk(6)
            emit_block(2, 0)
            emit_qk(3); emit_qk(7)
            emit_block(3, 0)
            # qb k's proj tiles ride one-per-block through qb k+1, and v
            # time-group k lands right after block (0,k) -- both keep the
            # PE fed while ScalarE chews that block's exps. Block (3,k) is
            # normed during block (0,k+1) via the AV deferral, so att qb-k
            # is complete before its first proj tile; v group k is emitted
            # before block (1,k) where block (0,k)'s deferred AVs run.
            for qb in range(1, QB):
                for hp in range(4):
                    emit_block(hp, qb)
                    if hp == 0:
                        emit_v(qb)
                    emit_proj_group([(qb - 1) * 4 + hp])
            flush_block()
            emit_proj_group(range(12, 16))
        else:
            if 1 in phases:
                for jt in range(CT):
                    emit_qk(jt)
                for tg in range(TT // 4):
                    emit_v(tg)
            if 2 in phases:
                for qb in range(QB):
                    for hp in range(HG // 2):
                        emit_block(hp, qb)
                        flush_block()
                    emit_proj_group(range(qb * 4, qb * 4 + 4))
            else:
                emit_proj_group(range(TT))


def _get_compiled():
    global _COMPILED
    if _COMPILED is None:
        _COMPILED = _build_nc()
    return _COMPILED


def _dr_pack(a):
    """[K, N] -> [K/2, 2*N] fp8 DoubleRow plane packing:
    out[tp*128+p, pl*N+n] = a[(2*tp+pl)*128 + p, n]."""
    K, N = a.shape
    return np.ascontiguousarray(
        a.reshape(K // 256, 2, 128, N).transpose(0, 2, 1, 3).reshape(K // 2, 2 * N))


def _make_in_maps(x, w_qkv, b_qkv, w_proj):
    in_maps = []
    for c in range(N_CORES):
        b, g = c // 2, c % 2
        s = slice(g * GC, (g + 1) * GC)
        xt = np.ascontiguousarray(x[b].T)
        w_qk = np.concatenate(
            [w_qkv[:, s], w_qkv[:, C + g * GC:C + (g + 1) * GC]], axis=1)
        in_maps.append({
            "xT": xt.astype(_bf16np),
            "x_dr": _dr_pack(xt.astype(_fp8np)),
            "w_dr": _dr_pack((w_qk * WSCALE).astype(_fp8np)),
            "w_v": np.ascontiguousarray(
                w_qkv[:, 2 * C + g * GC:2 * C + (g + 1) * GC]).astype(_bf16np),
            "b_qk": np.ascontiguousarray(
                np.concatenate([b_qkv[s], b_qkv[C + g * GC:C + (g + 1) * GC]])
            ) * np.float32(WSCALE),
            "b_v": np.ascontiguousarray(b_qkv[2 * C + g * GC:2 * C + (g + 1) * GC]),
            "w_pr": np.ascontiguousarray(w_proj[g * GC:(g + 1) * GC, :]).astype(_bf16np),
        })
    return in_maps


_RUNNER = None


def _get_runner():
    """Compile once, cache the jitted shard_map executable across calls."""
    global _RUNNER
    if _RUNNER is not None:
        return _RUNNER
    import jax
    from jax.sharding import Mesh, PartitionSpec, NamedSharding
    from jax.experimental.shard_map import shard_map
    from concourse.bass2jax import (_bass_exec_p, install_neuronx_cc_hook,
                                    partition_id_tensor)

    nc = _get_compiled()
    install_neuronx_cc_hook()
    partition_name = nc.partition_id_tensor.name if nc.partition_id_tensor else None
    in_names, out_names, out_avals, zero_outs = [], [], [], []
    for alloc in nc.m.functions[0].allocations:
        if not isinstance(alloc, mybir.MemoryLocationSet):
            continue
        name = alloc.memorylocations[0].name
        if alloc.kind == "ExternalInput":
            if name != partition_name:
                in_names.append(name)
        elif alloc.kind == "ExternalOutput":
            out_names.append(name)
            out_avals.append(jax.core.ShapedArray(tuple(alloc.tensor_shape),
                                                  mybir.dt.np(alloc.dtype)))
            zero_outs.append(np.zeros(tuple(alloc.tensor_shape),
                                      mybir.dt.np(alloc.dtype)))
    all_in = list(in_names) + list(out_names)
    if partition_name:
        all_in.append(partition_name)

    def _body(*args):
        ops = list(args)
        if partition_name:
            ops.append(partition_id_tensor())
        return tuple(_bass_exec_p.bind(
            *ops, out_avals=tuple(out_avals), in_names=tuple(all_in),
            out_names=tuple(out_names), lowering_input_output_aliases=(),
            sim_require_finite=True, sim_require_nnan=True, nc=nc))

    devices = jax.devices()[:N_CORES]
    mesh = Mesh(np.asarray(devices), ("core",))
    sharded = jax.jit(shard_map(
        _body, mesh=mesh,
        in_specs=(PartitionSpec("core"),) * (len(in_names) + len(out_avals)),
        out_specs=(PartitionSpec("core"),) * len(out_avals), check_rep=False),
        keep_unused=True)
    sharding = NamedSharding(mesh, PartitionSpec("core"))
    _RUNNER = (sharded, in_names, zero_outs, sharding, out_avals, out_names)
    return _RUNNER


def _execute(in_maps):
    import jax
    sharded, in_names, zero_outs, sharding, out_avals, out_names = _get_runner()
    ci = [jax.device_put(
        np.concatenate([np.asarray(in_maps[c][n]) for c in range(N_CORES)], axis=0),
        sharding) for n in in_names]
    cz = [jax.device_put(np.zeros((N_CORES * z.shape[0], *z.shape[1:]), z.dtype),
                         sharding) for z in zero_outs]
    outs = sharded(*ci, *cz)
    yi = out_names.index("y")
    return np.asarray(outs[yi]).reshape(N_CORES, *out_avals[yi].shape)


def run(x, w_qkv, b_qkv, w_proj, b_proj, trace=False):
    in_maps = _make_in_maps(np.asarray(x, dtype=np.float32),
                            np.asarray(w_qkv, dtype=np.float32),
                            np.asarray(b_qkv, dtype=np.float32),
                            np.asarray(w_proj, dtype=np.float32))
    y8 = _execute(in_maps)
    out = np.empty((B, T, C), dtype=np.float32)
    bp = np.asarray(b_proj, dtype=np.float32)
    for b in range(B):
        out[b] = y8[2 * b] + y8[2 * b + 1] + bp
    return out


def kernel(x, w_qkv, b_qkv, w_proj, b_proj):
    return run(x, w_qkv, b_qkv, w_proj, b_proj)


# revision 22
# speedup vs baseline: 1.3036x; 1.3036x over previous
"""Trainium2 Bass kernel for causal self-attention (muP scaling).

Full-input contract: kernel(**inputs) takes the complete tensors and returns
the complete [B, T, C] output. Internally the work is split over 8 NeuronCores
as (batch b = core//2) x (head-group g = core%2, 8 heads each):

  - each core computes q,k,v for its batch restricted to its 8 heads,
    runs causal attention for those heads, and multiplies by the matching
    512-row slice of w_proj, producing a partial [T, C] output.
  - the host sums the two partials per batch and adds b_proj. No on-device
    collectives are needed.

Layout trick: the host passes x[b].T (i.e. [C, T]) so that
  - qT,kT ([dim, t]) come from matmuls with the weight slice as the
    stationary operand and xT as the moving operand,
  - v ([t, dim]) comes from matmuls with xT tiles as the stationary operand,
so no on-chip transposes are needed anywhere.

Attention runs per head PAIR: the even head lives at SBUF partitions 0:64
and the odd head at 64:128 of the qkT tiles, so the two K=64 score matmuls
occupy disjoint PE row-groups (concurrent in the systolic array) and write
the two banks of one [128, 1024] PSUM tile, which a single ScalarE exp
drains (2-segment strided AP; muP scale 1/64 folded into the activation
scale; no max-subtraction - logits are ~N(0, 0.13) so exp cannot overflow).
Causal masking is a 0/1 upper-triangular multiply on diagonal-crossing
tiles only; fully-invalid tiles are never computed. attT-out[d, tq]
accumulates v_aug.T @ expT where v_aug carries an appended ones column, so
row 64 of the accumulator is the softmax denominator for free.
Normalization: reciprocal of that row, partition-broadcast on GpSimd, one
fused multiply while copying PSUM->SBUF. The normalized attention output
lands directly in [c, t] layout - the stationary-operand layout the final
projection wants. Attention blocks iterate tq-block-outer so each finished
tq column group's output projection interleaves with the next block's
(ScalarE-paced) attention. Activations ride bf16 (inputs pre-cast on the
host); measured end-to-end error vs the fp32 reference is ~4e-3 relative.
"""

import sys

if "/opt/trn_rl_repo" not in sys.path:
    sys.path.insert(0, "/opt/trn_rl_repo")

import numpy as np
import ml_dtypes

import concourse.bass as bass
import concourse.mybir as mybir
import concourse.tile as tile
from concourse import bacc
from concourse.bass_utils import run_bass_kernel_spmd
from concourse.masks import make_upper_triangular

# Problem shape (hardcoded per contract).
B, T, C, H = 4, 2048, 1024, 16
HD = C // H            # 64
N_CORES = 8
HG = H // 2            # 8 heads per core
GC = HG * HD           # 512 columns of q/k/v per core
P = 128                # SBUF partitions
CT = C // P            # 8 contraction tiles over C
TT = T // P            # 16 time tiles of 128
QB = 4                 # tq blocks
QW = T // QB           # 512 wide
KT = T // P            # 16 tk tiles

_bf16np = ml_dtypes.bfloat16
F32 = mybir.dt.float32
F32R = mybir.dt.float32r
BF16 = mybir.dt.bfloat16

_COMPILED = None


def _r(ap):
    """Reinterpret an fp32 AP as float32r for full-rate PE matmuls."""
    return ap.bitcast(F32R)


def _build_nc(reps=1, phases=(1, 2, 3), p2mode="full", pipeline=False, all_bf16=True, exp_split=False):
    nc = bacc.Bacc("TRN2", target_bir_lowering=False, debug=False,
                   num_devices=N_CORES)

    adt = BF16 if all_bf16 else F32
    xT = nc.dram_tensor("xT", [C, T], adt, kind="ExternalInput").ap()
    w_qk = nc.dram_tensor("w_qk", [C, 2 * GC], adt, kind="ExternalInput").ap()
    w_v = nc.dram_tensor("w_v", [C, GC], adt, kind="ExternalInput").ap()
    b_qk = nc.dram_tensor("b_qk", [2 * GC], F32, kind="ExternalInput").ap()
    b_v = nc.dram_tensor("b_v", [GC], F32, kind="ExternalInput").ap()
    w_pr = nc.dram_tensor("w_pr", [GC, C], BF16, kind="ExternalInput").ap()
    y = nc.dram_tensor("y", [T, C], F32, kind="ExternalOutput").ap()

    with tile.TileContext(nc) as tc:
        for _ in range(reps):
            _emit(nc, tc, xT, w_qk, w_v, b_qk, b_v, w_pr, y, phases=phases, p2mode=p2mode, pipeline=pipeline, all_bf16=all_bf16, exp_split=exp_split)
    nc.finalize()
    return nc


def _emit(nc, tc, xT, w_qk, w_v, b_qk, b_v, w_pr, y, phases=(1, 2, 3), p2mode="full", pipeline=False, all_bf16=True, exp_split=False):
    from contextlib import ExitStack

    ctx = ExitStack()
    with ctx:
        persist = ctx.enter_context(tc.tile_pool(name="persist", bufs=1))

        # ---- constants -------------------------------------------------
        tri = persist.tile([P, P], BF16, tag="tri")     # 0/1, 1 iff j >= i
        make_upper_triangular(nc, tri[:, :], val=1.0, diag=True)

        bqk_sb = persist.tile([P, CT], F32, tag="bqk")  # [128, 8] col jt
        nc.sync.dma_start(
            out=bqk_sb[:, :],
            in_=bass.AP(tensor=b_qk.tensor, offset=0, ap=[[1, P], [P, CT]]),
        )
        bv_sb = persist.tile([P, GC], F32, tag="bv")
        nc.gpsimd.dma_start(
            out=bv_sb[:, :],
            in_=bass.AP(tensor=b_v.tensor, offset=0, ap=[[0, P], [1, GC]]),
        )

        # ---- persistent activation buffers ----------------------------
        mdt = BF16 if all_bf16 else F32R
        qkT = [persist.tile([P, T], mdt, name=f"qkT{j}", tag=f"qkT{j}") for j in range(CT)]
        v_sb = [persist.tile([P, HG, HD + 1], BF16, name=f"v{t}", tag=f"v{t}")
                for t in range(TT)]

        # ================= phase 1: qkv projections ====================
        with tc.tile_pool(name="xT", bufs=1) as xp:
            xts = [xp.tile([P, T], mdt, name=f"xT{ct}", tag=f"xT{ct}")
                   for ct in range(CT)]

            with tc.tile_pool(name="wqk", bufs=1) as wp, \
                 tc.tile_pool(name="ps1", bufs=8, space="PSUM") as ps1:
                wts = [wp.tile([P, 2 * GC], mdt, name=f"wqk{ct}", tag=f"wqk{ct}")
                       for ct in range(CT)]
                # interleave x/w loads so the first accumulation step's
                # operands (x0, w0) land before the tail of either stream
                for ct in range(CT):
                    nc.sync.dma_start(out=xts[ct][:, :],
                                      in_=xT[ct * P:(ct + 1) * P, :] if all_bf16
                                      else xT[ct * P:(ct + 1) * P, :].bitcast(F32R))
                    nc.sync.dma_start(out=wts[ct][:, :],
                                      in_=w_qk[ct * P:(ct + 1) * P, :] if all_bf16
                                      else w_qk[ct * P:(ct + 1) * P, :].bitcast(F32R))
                for jt in range(CT if 1 in phases else 0):
                    # ct-outer so the first matmuls only need tile ct=0 loaded
                    pss_ = [ps1.tile([P, QW], F32, name=f"ps1_{jt}_{tb}", tag="ps1")
                            for tb in range(QB)]
                    for ct in range(CT):
                        for tb in range(QB):
                            nc.tensor.matmul(
                                pss_[tb][:, :],
                                wts[ct][:, jt * P:(jt + 1) * P],
                                xts[ct][:, tb * QW:(tb + 1) * QW],
                                start=(ct == 0), stop=(ct == CT - 1),
                            )
                    for tb in range(QB):
                        nc.vector.tensor_scalar_add(
                            out=qkT[jt][:, tb * QW:(tb + 1) * QW],
                            in0=pss_[tb][:, :],
                            scalar1=bqk_sb[:, jt:jt + 1],
                        )

            with tc.tile_pool(name="wv", bufs=1) as wvp, \
                 tc.tile_pool(name="ps1v", bufs=8, space="PSUM") as ps1v:
                wvts = []
                for ct in range(CT):
                    wvt = wvp.tile([P, GC], mdt, name=f"wv{ct}", tag=f"wv{ct}")
                    nc.sync.dma_start(out=wvt[:, :],
                                      in_=w_v[ct * P:(ct + 1) * P, :] if all_bf16
                                      else w_v[ct * P:(ct + 1) * P, :].bitcast(F32R))
                    wvts.append(wvt)
                for tg in range(TT // 4 if 1 in phases else 0):
                    pss_ = [ps1v.tile([P, GC], F32, name=f"ps1v_{tg}_{i}", tag="ps1v")
                            for i in range(4)]
                    for ct in range(CT):
                        for i in range(4):
                            tt = tg * 4 + i
                            nc.tensor.matmul(
                                pss_[i][:, :],
                                xts[ct][:, tt * P:(tt + 1) * P],
                                wvts[ct][:, :],
                                start=(ct == 0), stop=(ct == CT - 1),
                            )
                    for i in range(4):
                        tt = tg * 4 + i
                        nc.vector.tensor_add(
                            out=v_sb[tt][:, :, 0:HD],
                            in0=pss_[i][:, :].rearrange("p (h e) -> p h e", e=HD),
                            in1=bv_sb[:, :].rearrange("p (h e) -> p h e", e=HD),
                        )
                        nc.vector.memset(v_sb[tt][:, :, HD:HD + 1], 1.0)

        # ================= phase 2: attention ==========================
        # Opened after the xT pool closes so its SBUF space is reused.
        ph23 = ctx.enter_context(tc.tile_pool(name="ph23", bufs=1))
        att = [ph23.tile([P, T], BF16, name=f"att{j}", tag=f"att{j}") for j in range(CT // 2)]
        if p2mode in ("av_only", "scores_av"):
            dummy_ex = ph23.tile([P, 2 * QW], BF16, tag="dummy_ex")
            nc.vector.memset(dummy_ex[:, :], 0.5)
        if p2mode != "full":
            for j in range(CT // 2):
                nc.vector.memset(att[j][:, :], 0.01)
        wpr = [ph23.tile([P, C], BF16, name=f"wpr{j}", tag=f"wpr{j}") for j in range(CT // 2)]
        for ct in range(CT // 2):
            nc.sync.dma_start(out=wpr[ct][:, :], in_=w_pr[ct * P:(ct + 1) * P, :])

        do_scores = p2mode in ("full", "scores_only", "scores_exp", "scores_av")
        do_exp = p2mode in ("full", "scores_exp")
        do_av = p2mode in ("full", "av_only", "scores_av")
        do_norm = p2mode == "full"

        with tc.tile_pool(name="expp", bufs=20) as expp, \
             tc.tile_pool(name="nrm", bufs=4) as nrm, \
             tc.tile_pool(name="ysb", bufs=3) as yp, \
             tc.tile_pool(name="ps_s", bufs=2, space="PSUM") as pss, \
             tc.tile_pool(name="ps_o", bufs=2, space="PSUM") as pso, \
             tc.tile_pool(name="ps3", bufs=2, space="PSUM") as ps3:
            # Head PAIRS: even head at partitions 0:64, odd at 64:128 of the
            # qkT tiles. The two score matmuls use disjoint PE row-groups and
            # run concurrently; their outputs land in the two banks of one
            # [128, 1024] PSUM tile so a single ACT exp drains both.
            #
            # Software pipeline across (pair, block) iterations: the AV
            # matmuls of block k-1 are interleaved tile-by-tile with the
            # score matmuls of block k, so the PE never sits waiting for
            # ScalarE to finish the exps of the block it just scored.
            blocks = []
            if 2 in phases:
                for qb in range(QB):
                    for hp in range(HG // 2):
                        tiles = [(kt, 0, False) for kt in range(4 * qb)]
                        tiles += [(4 * qb + a, P * a, True) for a in range(4)]
                        blocks.append((hp, qb, tiles))

            def emit_scores(hp, q0, kt, off, crossing):
                n = QW - off
                qT_t, kT_t = qkT[hp], qkT[CT // 2 + hp]
                ex = expp.tile([P, 2 * QW], BF16, tag="exp")
                if not do_scores:
                    return dummy_ex if do_av else ex
                ps = pss.tile([P, 2 * QW], F32, tag="scores")
                nc.tensor.matmul(
                    ps[:, 0:n],
                    kT_t[0:HD, kt * P:(kt + 1) * P],
                    qT_t[0:HD, q0 + off:q0 + QW],
                    start=True, stop=True,
                )
                nc.tensor.matmul(
                    ps[:, QW:QW + n],
                    kT_t[HD:P, kt * P:(kt + 1) * P],
                    qT_t[HD:P, q0 + off:q0 + QW],
                    start=True, stop=True,
                )
                if do_exp:
                    if exp_split:
                        nc.scalar.activation(
                            out=ex[:, 0:n], in_=ps[:, 0:n],
                            func=mybir.ActivationFunctionType.Exp,
                            scale=1.0 / HD,
                        )
                        nc.scalar.activation(
                            out=ex[:, QW:QW + n], in_=ps[:, QW:QW + n],
                            func=mybir.ActivationFunctionType.Exp,
                            scale=1.0 / HD,
                        )
                    else:
                        # one exp over both heads: 2-segment strided view
                        ps2 = ps[:, :].rearrange("p (s q) -> p s q", s=2)
                        ex2 = ex[:, :].rearrange("p (s q) -> p s q", s=2)
                        nc.scalar.activation(
                            out=ex2[:, :, 0:n], in_=ps2[:, :, 0:n],
                            func=mybir.ActivationFunctionType.Exp,
                            scale=1.0 / HD,
                        )
                    if crossing:
                        # diagonal-crossing tile: triangle on cols 0:128
                        nc.vector.tensor_mul(
                            out=ex[:, 0:P], in0=ex[:, 0:P], in1=tri[:, :])
                        nc.vector.tensor_mul(
                            out=ex[:, QW:QW + P], in0=ex[:, QW:QW + P],
                            in1=tri[:, :])
                else:
                    # timing diagnostics: tiny consumer so the score matmuls
                    # aren't dead code
                    nc.vector.tensor_copy(out=ex[:, 0:2].bitcast(F32),
                                          in_=ps[:, 0:1])
                    if do_av:
                        ex = dummy_ex
                return ex

            def emit_av(st, i):
                (hp, q0, accs, exps) = st
                kt, off, n, ex = exps[i]
                last = i == len(exps) - 1
                nc.tensor.matmul(
                    accs[0][0:HD + 1, off:QW],
                    v_sb[kt][:, 2 * hp, :],
                    ex[:, 0:n],
                    start=(i == 0), stop=last,
                    skip_group_check=True,
                )
                nc.tensor.matmul(
                    accs[1][0:HD + 1, off:QW],
                    v_sb[kt][:, 2 * hp + 1, :],
                    ex[:, QW:QW + n],
                    start=(i == 0), stop=last,
                    skip_group_check=True,
                )

            def emit_norm(st):
                (hp, q0, accs, exps) = st
                for half, acc in ((0, accs[0]), (1, accs[1])):
                    r0 = half * HD
                    if do_norm:
                        rec = nrm.tile([P, QW], F32, tag="rec")
                        nc.vector.reciprocal(out=rec[0:1, :],
                                             in_=acc[HD:HD + 1, :])
                        bc = nrm.tile([P, QW], F32, tag="bc")
                        nc.gpsimd.partition_broadcast(
                            bc[0:HD, :], rec[0:1, :], channels=HD)
                        nc.vector.tensor_mul(
                            out=att[hp][r0:r0 + HD, q0:q0 + QW],
                            in0=acc[0:HD, :],
                            in1=bc[0:HD, :],
                        )
                    else:
                        nc.vector.tensor_copy(
                            out=att[hp][r0:r0 + HD, q0:q0 + QW],
                            in_=acc[0:HD, :])

            def emit_proj_group(tts):
                if 3 not in phases:
                    return
                for tt in tts:
                    ysb = yp.tile([P, C], F32, tag="y")
                    for nb in range(2):
                        ps = ps3.tile([P, QW], F32, tag="ps3")
                        for ct in range(CT // 2):
                            nc.tensor.matmul(
                                ps[:, :],
                                att[ct][:, tt * P:(tt + 1) * P],
                                wpr[ct][:, nb * QW:(nb + 1) * QW],
                                start=(ct == 0), stop=(ct == CT // 2 - 1),
                            )
                        nc.vector.tensor_copy(
                            out=ysb[:, nb * QW:(nb + 1) * QW], in_=ps[:, :])
                    nc.sync.dma_start(out=y[tt * P:(tt + 1) * P, :], in_=ysb[:, :])

            pend = None  # previous block waiting for its AV matmuls
            done_qb = -1
            for hp, qb, tiles in blocks:
                if qb != done_qb and done_qb >= 0:
                    # tq columns of the finished qb group are final in att:
                    # overlap their output projection with this qb's attention
                    if pend is not None and do_av:
                        for j in range(len(pend[3])):
                            emit_av(pend, j)
                        emit_norm(pend)
                        pend = None
                    emit_proj_group(range(done_qb * 4, done_qb * 4 + 4))
                done_qb = qb
                q0 = qb * QW
                acc_e = pso.tile([P, QW], F32, name=f"acc_e{hp}_{qb}", tag="acc")
                acc_o = pso.tile([P, QW], F32, name=f"acc_o{hp}_{qb}", tag="acc")
                exps = []
                np_prev = len(pend[3]) if pend is not None else 0
                for i, (kt, off, crossing) in enumerate(tiles):
                    ex = emit_scores(hp, q0, kt, off, crossing)
                    exps.append((kt, off, QW - off, ex))
                    if do_av and pend is not None:
                        # drain previous block's AVs at matching pace
                        lo = i * np_prev // len(tiles)
                        hi = (i + 1) * np_prev // len(tiles)
                        for j in range(lo, hi):
                            emit_av(pend, j)
                if pend is not None:
                    if do_av:
                        emit_norm(pend)
                    pend = None
                if do_av:
                    st = (hp, q0, (acc_e, acc_o), exps)
                    if pipeline:
                        pend = st
                    else:
                        for j in range(len(exps)):
                            emit_av(st, j)
                        emit_norm(st)
            if pend is not None and do_av:
                for j in range(len(pend[3])):
                    emit_av(pend, j)
                emit_norm(pend)
            if 2 in phases:
                emit_proj_group(range(done_qb * 4, done_qb * 4 + 4))
            else:
                emit_proj_group(range(TT))


def _get_compiled():
    global _COMPILED
    if _COMPILED is None:
        _COMPILED = _build_nc()
    return _COMPILED


def _make_in_maps(x, w_qkv, b_qkv, w_proj, all_bf16=True):
    adt = _bf16np if all_bf16 else np.float32
    in_maps = []
    for c in range(N_CORES):
        b, g = c // 2, c % 2
        s = slice(g * GC, (g + 1) * GC)
        in_maps.append({
            "xT": np.ascontiguousarray(x[b].T).astype(adt),
            "w_qk": np.ascontiguousarray(
                np.concatenate([w_qkv[:, s], w_qkv[:, C + g * GC:C + (g + 1) * GC]],
                               axis=1)).astype(adt),
            "w_v": np.ascontiguousarray(
                w_qkv[:, 2 * C + g * GC:2 * C + (g + 1) * GC]).astype(adt),
            "b_qk": np.ascontiguousarray(
                np.concatenate([b_qkv[s], b_qkv[C + g * GC:C + (g + 1) * GC]])),
            "b_v": np.ascontiguousarray(b_qkv[2 * C + g * GC:2 * C + (g + 1) * GC]),
            "w_pr": np.ascontiguousarray(w_proj[g * GC:(g + 1) * GC, :]).astype(_bf16np),
        })
    return in_maps


_RUNNER = None


def _get_runner():
    """Compile once, cache the jitted shard_map executable across calls."""
    global _RUNNER
    if _RUNNER is not None:
        return _RUNNER
    import jax
    from jax.sharding import Mesh, PartitionSpec, NamedSharding
    from jax.experimental.shard_map import shard_map
    from concourse.bass2jax import (_bass_exec_p, install_neuronx_cc_hook,
                                    partition_id_tensor)

    nc = _get_compiled()
    install_neuronx_cc_hook()
    partition_name = nc.partition_id_tensor.name if nc.partition_id_tensor else None
    in_names, out_names, out_avals, zero_outs = [], [], [], []
    for alloc in nc.m.functions[0].allocations:
        if not isinstance(alloc, mybir.MemoryLocationSet):
            continue
        name = alloc.memorylocations[0].name
        if alloc.kind == "ExternalInput":
            if name != partition_name:
                in_names.append(name)
        elif alloc.kind == "ExternalOutput":
            out_names.append(name)
            out_avals.append(jax.core.ShapedArray(tuple(alloc.tensor_shape),
                                                  mybir.dt.np(alloc.dtype)))
            zero_outs.append(np.zeros(tuple(alloc.tensor_shape),
                                      mybir.dt.np(alloc.dtype)))
    all_in = list(in_names) + list(out_names)
    if partition_name:
        all_in.append(partition_name)

    def _body(*args):
        ops = list(args)
        if partition_name:
            ops.append(partition_id_tensor())
        return tuple(_bass_exec_p.bind(
            *ops, out_avals=tuple(out_avals), in_names=tuple(all_in),
            out_names=tuple(out_names), lowering_input_output_aliases=(),
            sim_require_finite=True, sim_require_nnan=True, nc=nc))

    devices = jax.devices()[:N_CORES]
    mesh = Mesh(np.asarray(devices), ("core",))
    sharded = jax.jit(shard_map(
        _body, mesh=mesh,
        in_specs=(PartitionSpec("core"),) * (len(in_names) + len(out_avals)),
        out_specs=(PartitionSpec("core"),) * len(out_avals), check_rep=False),
        keep_unused=True)
    sharding = NamedSharding(mesh, PartitionSpec("core"))
    _RUNNER = (sharded, in_names, zero_outs, sharding, out_avals, out_names)
    return _RUNNER


def _execute(in_maps):
    import jax
    sharded, in_names, zero_outs, sharding, out_avals, out_names = _get_runner()
    ci = [jax.device_put(
        np.concatenate([np.asarray(in_maps[c][n]) for c in range(N_CORES)], axis=0),
        sharding) for n in in_names]
    cz = [jax.device_put(np.zeros((N_CORES * z.shape[0], *z.shape[1:]), z.dtype),
                         sharding) for z in zero_outs]
    outs = sharded(*ci, *cz)
    yi = out_names.index("y")
    return np.asarray(outs[yi]).reshape(N_CORES, *out_avals[yi].shape)


def run(x, w_qkv, b_qkv, w_proj, b_proj, trace=False):
    in_maps = _make_in_maps(np.asarray(x, dtype=np.float32),
                            np.asarray(w_qkv, dtype=np.float32),
                            np.asarray(b_qkv, dtype=np.float32),
                            np.asarray(w_proj, dtype=np.float32))
    y8 = _execute(in_maps)
    out = np.empty((B, T, C), dtype=np.float32)
    bp = np.asarray(b_proj, dtype=np.float32)
    for b in range(B):
        out[b] = y8[2 * b] + y8[2 * b + 1] + bp
    return out


def kernel(x, w_qkv, b_qkv, w_proj, b_proj):
    return run(x, w_qkv, b_qkv, w_proj, b_proj)

